# revision 61
# baseline (speedup 1.0000x reference)
"""TRN2 Bass kernel for nn_LogDomainResNet.

The reference network is a signed-log-domain encoding of a plain
real-domain tanh ResNet:

    v0      = sign_x * exp(log_abs_x)
    v_{i+1} = tanh(v_i @ W_i + b_i) + v_i        (7 inner layers)
    t       = v_7 @ W_final
    out     = stack([sign(t), log|t|])

All slog plumbing (per-row max, exp/log per layer) cancels exactly, so the
kernel computes in the real domain. Values stay bounded (|v| < 32), so fp32
range is never an issue.

Precision (1.5-pass scheme): each matmul runs as one fp16 pass plus one
fp8e4 DoubleRow correction accumulated into the same PSUM group.
  main:  vh @ (Wh * 2^12)            vh = fp16(v), Wh = fp16(W)
  corr:  (vl*2^8) @ (Wh8*2^4)  +  vh8 @ (Wl8*2^12)     [one DR instr/chunk]
All terms carry 2^12, so tanh reads PSUM with scale=2^-12. Effective
operand precision ~15-16 bits; the exact v is carried in an fp32 residual
tensor vf (updated in place), from which vh/vl are re-derived each layer.

Layout: activations live transposed ([feature -> partitions, batch -> free]);
the host precomputes v0's whole operand set (vh fp16 / vl bf16 / fp8 lane
pair, pre-transposed) and the bias in its SBUF layout, so the kernel has no
input stage at all — layer-0 weights and the v0 set stream in
chunk-interleaved and matmuls start immediately.  w8 lane 0 is derived
on-chip from the fp16 weights on the otherwise idle GpSimd engine.  The
final layer swaps operands (stationary = vh tile) to produce t in natural
[batch, feature] layout (sign/abs as DVE bitwise ops), so outputs DMA out
contiguously.

Sharding: data-parallel over the batch axis, 1024 rows per core x 8 cores.
"""

import numpy as np
import ml_dtypes

_B, _D, _NL = 8192, 1024, 8  # batch, width, layers (7 inner + final)
_NCORES = 8
_BP = _B // _NCORES          # batch rows per core
_P = 128
_KC = _D // _P               # contraction chunks per matmul
_BT = _BP // _P              # batch tiles (final stage)
_BCH = 512                   # PSUM free dim
_NBC = _BP // _BCH           # batch chunks per layer pass
_NT = _D // _P               # out-feature tiles per layer
_SC = 4096.0                 # 2^12 PSUM scale carried by both passes

_cached_nc = None
last_results = None  # BassKernelResults from the most recent run (for test.py)


def _build():
    import concourse.mybir as mybir
    from concourse import bacc
    from concourse.tile import TileContext

    f32, f16 = mybir.dt.float32, mybir.dt.float16
    bf16, f8 = mybir.dt.bfloat16, mybir.dt.float8e4
    i32 = mybir.dt.int32
    AF = mybir.ActivationFunctionType
    ALU = mybir.AluOpType
    DR = mybir.MatmulPerfMode.DoubleRow

    nc = bacc.Bacc("TRN2", target_bir_lowering=False, debug=False)
    # host-prepared v0 operand set, pre-transposed to [D, BP] / [D, 2, BP]
    d_vl = nc.dram_tensor("vl0", [_D, _BP], bf16, kind="ExternalInput")
    d_vh = nc.dram_tensor("vh0", [_D, _BP], f16, kind="ExternalInput")
    d_v8 = nc.dram_tensor("v80", [_D, 2, _BP], f8, kind="ExternalInput")
    d_wh = nc.dram_tensor("wh", [_NL, _D, _D], f16, kind="ExternalInput")
    d_w8 = nc.dram_tensor("w8l", [_NL, _D, _D], f8, kind="ExternalInput")
    d_bias = nc.dram_tensor("bias", [_P, (_NL - 1) * _NT], f32, kind="ExternalInput")
    d_out = nc.dram_tensor("out", [2, _BP, _D], f32, kind="ExternalOutput")

    with TileContext(nc) as tc:
        with (
            tc.tile_pool(name="const", bufs=1) as constp,
            tc.tile_pool(name="w", bufs=2) as wp,
            tc.tile_pool(name="w8", bufs=2) as w8p,
            tc.tile_pool(name="vh", bufs=2) as vhp,
            tc.tile_pool(name="v8", bufs=2) as v8p,
            tc.tile_pool(name="vf", bufs=1) as vfp,
            tc.tile_pool(name="inp", bufs=1) as inp,
            tc.tile_pool(name="tmp", bufs=4) as tmp,
            tc.tile_pool(name="ps", bufs=4, space="PSUM") as ps,
        ):
            bias_sb = constp.tile([_P, (_NL - 1) * _NT], f32)

            # ---- input: v0 operand set DMA'd straight in (host-prepared) ----
            vf = vfp.tile([_P, _KC, _BP], f32, tag="vf")
            vh = vhp.tile([_P, _KC, _BP], f16, tag="vh")
            v8 = v8p.tile([_P, _KC, 2, _BP], f8, tag="v8")
            # layer-0 weights interleaved with the v0 set, chunk by chunk, so
            # the first matmuls start as soon as chunk 0 lands.  w8 lane 0
            # (Wh8 = fp8(Wh * 2^4)) is derived on-chip from the fp16 weights;
            # only lane 1 (Wl8) ships from the host.
            ws0 = wp.tile([_P, _KC, _D], f16, tag="w", name="ws0")
            w8s0 = w8p.tile([_P, _KC, 2, _D], f8, tag="w8", name="w8s0")
            vl0 = inp.tile([_P, _KC, _BP], bf16, tag="vl0")
            nc.sync.dma_start(bias_sb[:], d_bias[:, :])
            for c in range(_KC):
                csl = slice(c * _P, (c + 1) * _P)
                nc.sync.dma_start(ws0[:, c, :], d_wh[0, csl, :])
                nc.sync.dma_start(vh[:, c, :], d_vh[csl, :])
                if c == 0:
                    # PE warm-up: the HAM clock gate only reaches full rate
                    # after ~3us of continuous busy; layer 0 is DMA-paced, so
                    # prime the ramp with throwaway matmuls on the first
                    # weight chunk (result never read)
                    warm = ps.tile([_P, _BCH], f32, tag="warm", name="warmps")
                    for wi in range(12):
                        nc.tensor.matmul(
                            warm[:], ws0[:, 0, :_P], ws0[:, 0, :_BCH],
                            start=True, stop=True,
                        )
                nc.sync.dma_start(v8[:, c, :, :], d_v8[csl, :, :])
                nc.sync.dma_start(w8s0[:, c, 1, :], d_w8[0, csl, :])
                if c >= _KC // 2:
                    # trailing half: also pull in vl0 for the early epilogues
                    # (they gate PSUM-bank recycling in layer 0)
                    vc = c - _KC // 2
                    vcsl = slice(vc * _P, (vc + 1) * _P)
                    nc.sync.dma_start(vl0[:, vc, :], d_vl[vcsl, :])
                nc.gpsimd.tensor_scalar_mul(
                    out=w8s0[:, c, 0, :], in0=ws0[:, c, :], scalar1=0.00390625
                )
            for c in range(_KC // 2, _KC):
                csl = slice(c * _P, (c + 1) * _P)
                nc.sync.dma_start(vl0[:, c, :], d_vl[csl, :])

            # ---- 7 inner layers: v = tanh(v @ W + b) + v ----
            for i in range(_NL - 1):
                if i == 0:
                    ws, w8s = ws0, w8s0
                else:
                    ws = wp.tile([_P, _KC, _D], f16, tag="w")
                    nc.sync.dma_start(
                        ws[:], d_wh[i].rearrange("(c p) n -> p c n", p=_P)
                    )
                    w8s = w8p.tile([_P, _KC, 2, _D], f8, tag="w8")
                    nc.sync.dma_start(
                        w8s[:, :, 1, :],
                        d_w8[i].rearrange("(c p) n -> p c n", p=_P),
                    )
                    for c in range(_KC):
                        nc.gpsimd.tensor_scalar_mul(
                            out=w8s[:, c, 0, :], in0=ws[:, c, :], scalar1=0.00390625
                        )
                vh_new = vhp.tile([_P, _KC, _BP], f16, tag="vh")
                v8_new = v8p.tile([_P, _KC, 2, _BP], f8, tag="v8")
                for n in range(_NT):
                    nsl = slice(n * _P, (n + 1) * _P)
                    pts = [
                        ps.tile([_P, _BCH], f32, tag="mm", name=f"pt{i}_{n}_{bc}")
                        for bc in range(_NBC)
                    ]
                    # last group: finish bc0 (matmuls + epilogue) before bc1's
                    # matmuls so the next layer's last-needed chunk is ready
                    # sooner; other groups interleave bc to share DR weights
                    bc_waves = (
                        [[bc] for bc in range(_NBC)]
                        if n == _NT - 1 else [list(range(_NBC))]
                    )
                    for wave in bc_waves:
                        for c in range(_KC):
                            for bc in wave:
                                bsl = slice(bc * _BCH, (bc + 1) * _BCH)
                                nc.tensor.matmul(
                                    pts[bc][:], ws[:, c, nsl], vh[:, c, bsl],
                                    start=(c == 0), stop=False,
                                )
                        for c in range(_KC):
                            for bc in wave:
                                bsl = slice(bc * _BCH, (bc + 1) * _BCH)
                                nc.tensor.matmul(
                                    pts[bc][:], w8s[:, c, :, nsl], v8[:, c, :, bsl],
                                    start=False, stop=(c == _KC - 1),
                                    perf_mode=DR,
                                )
                        for bc in wave:
                            bsl = slice(bc * _BCH, (bc + 1) * _BCH)
                            u = tmp.tile([_P, _BCH], f32, tag="u", bufs=3)
                            nc.scalar.activation(
                                u[:], pts[bc][:], AF.Tanh, scale=1.0 / _SC,
                                bias=bias_sb[:, i * _NT + n : i * _NT + n + 1],
                            )
                            if i == 0:
                                # lazy vf init: vf = (u + vh0) + vl0
                                nc.vector.tensor_add(
                                    out=u[:], in0=u[:], in1=vh[:, n, bsl]
                                )
                                nc.vector.tensor_add(
                                    out=vf[:, n, bsl], in0=u[:], in1=vl0[:, n, bsl]
                                )
                            else:
                                nc.vector.tensor_add(
                                    out=vf[:, n, bsl], in0=u[:], in1=vf[:, n, bsl]
                                )
                            nc.scalar.activation(
                                vh_new[:, n, bsl], vf[:, n, bsl], AF.Identity
                            )
                            vlt = tmp.tile([_P, _BCH], bf16, tag="vl", bufs=3)
                            nc.vector.tensor_sub(
                                out=vlt[:], in0=vf[:, n, bsl], in1=vh_new[:, n, bsl]
                            )
                            nc.gpsimd.tensor_scalar_mul(
                                out=v8_new[:, n, 0, bsl], in0=vlt[:], scalar1=256.0
                            )
                            nc.scalar.activation(
                                v8_new[:, n, 1, bsl], vf[:, n, bsl], AF.Identity
                            )
                vh, v8 = vh_new, v8_new

            # ---- final layer: t = v @ W_f, out = [sign(t), log|t|] ----
            wf = wp.tile([_P, _KC, _D], f16, tag="w")
            nc.sync.dma_start(
                wf[:], d_wh[_NL - 1].rearrange("(c p) n -> p c n", p=_P)
            )
            w8f = w8p.tile([_P, _KC, 2, _D], f8, tag="w8")
            nc.sync.dma_start(
                w8f[:, :, 1, :],
                d_w8[_NL - 1].rearrange("(c p) n -> p c n", p=_P),
            )
            for c in range(_KC):
                nc.gpsimd.tensor_scalar_mul(
                    out=w8f[:, c, 0, :], in0=wf[:, c, :], scalar1=0.00390625
                )
            for bt in range(_BT):
                bsl = slice(bt * _P, (bt + 1) * _P)
                ptf = [
                    ps.tile([_P, _BCH], f32, tag="mm", name=f"ptf{bt}_{j}")
                    for j in range(_NBC)
                ]
                # j-interleaved so each (stationary) v-side tile is loaded
                # into the PE once per chunk, serving both j halves; the last
                # batch tile de-interleaves so j0 drains while j1 computes
                j_waves = (
                    [[j] for j in range(_NBC)]
                    if bt == _BT - 1 else [list(range(_NBC))]
                )
                for wave in j_waves:
                    for c in range(_KC):
                        for j in wave:
                            nsl = slice(j * _BCH, (j + 1) * _BCH)
                            nc.tensor.matmul(
                                ptf[j][:], vh[:, c, bsl], wf[:, c, nsl],
                                start=(c == 0), stop=False,
                            )
                    for c in range(_KC):
                        for j in wave:
                            nsl = slice(j * _BCH, (j + 1) * _BCH)
                            nc.tensor.matmul(
                                ptf[j][:], v8[:, c, :, bsl], w8f[:, c, :, nsl],
                                start=False, stop=(c == _KC - 1),
                                perf_mode=DR,
                            )
                    for j in wave:
                        nsl = slice(j * _BCH, (j + 1) * _BCH)
                        # sign/abs as DVE bitwise ops (sign = msb | 1.0f, abs
                        # = clear msb) so only the Ln runs on ACT — the
                        # trailing per-tile epilogue chain halves
                        sg = tmp.tile([_P, _BCH], f32, tag="sg", bufs=2)
                        nc.vector.tensor_scalar(
                            out=sg[:].bitcast(i32), in0=ptf[j][:].bitcast(i32),
                            scalar1=-0x80000000, scalar2=0x3F800000,
                            op0=ALU.bitwise_and, op1=ALU.bitwise_or,
                        )
                        ab = tmp.tile([_P, _BCH], f32, tag="ab", bufs=2)
                        nc.vector.tensor_scalar(
                            out=ab[:].bitcast(i32), in0=ptf[j][:].bitcast(i32),
                            scalar1=0x7FFFFFFF, scalar2=None,
                            op0=ALU.bitwise_and,
                        )
                        lg = tmp.tile([_P, _BCH], f32, tag="lg", bufs=2)
                        nc.scalar.activation(lg[:], ab[:], AF.Ln, scale=1.0 / _SC)
                        nc.sync.dma_start(d_out[0, bsl, nsl], sg[:])
                        nc.sync.dma_start(d_out[1, bsl, nsl], lg[:])
    nc.compile()
    return nc


def kernel(sign_x, log_abs_x, inner_kernels, final_kernel):
    global _cached_nc, last_results
    from concourse.bass_utils import run_bass_kernel_spmd

    if _cached_nc is None:
        _cached_nc = _build()
    nc = _cached_nc

    sign_x = np.asarray(sign_x, dtype=np.float32)
    log_abs_x = np.asarray(log_abs_x, dtype=np.float32)
    ik = np.asarray(inner_kernels, dtype=np.float32)
    fk = np.asarray(final_kernel, dtype=np.float32)
    e4 = ml_dtypes.float8_e4m3

    # host-side v0 operand set, transposed to [D, B]
    v0 = (sign_x * np.exp(log_abs_x)).T.astype(np.float32)
    vh0 = v0.astype(np.float16)
    vl0f = v0 - vh0.astype(np.float32)
    vl0 = vl0f.astype(ml_dtypes.bfloat16)
    v80 = np.empty((_D, 2, _B), dtype=e4)
    v80[:, 0, :] = np.clip(vl0f * 256.0, -240, 240).astype(e4)
    v80[:, 1, :] = np.clip(v0, -240, 240).astype(e4)

    W = np.concatenate([ik[:, :_D, :], fk[None]], axis=0)  # [8, 1024, 1024]
    Wh = W.astype(np.float16)
    Wl = W - Wh.astype(np.float32)
    w8l = np.ascontiguousarray(np.clip(Wl * _SC, -240, 240).astype(e4))
    WhS = np.ascontiguousarray(Wh.astype(np.float32) * _SC).astype(np.float16)
    # bias pre-arranged to the SBUF layout [p, (l t)] = [128, 56]
    bias = np.ascontiguousarray(
        ik[:, _D, :].reshape(_NL - 1, _NT, _P).transpose(2, 0, 1).reshape(_P, -1)
    )

    in_maps = []
    for cid in range(_NCORES):
        sl = slice(cid * _BP, (cid + 1) * _BP)
        in_maps.append({
            "vl0": np.ascontiguousarray(vl0[:, sl]),
            "vh0": np.ascontiguousarray(vh0[:, sl]),
            "v80": np.ascontiguousarray(v80[:, :, sl]),
            "wh": WhS,
            "w8l": w8l,
            "bias": bias,
        })

    last_results = run_bass_kernel_spmd(nc, in_maps, core_ids=list(range(_NCORES)))
    return np.concatenate([r["out"] for r in last_results.results], axis=1)


# revision 64
# speedup vs baseline: 1.0007x; 1.0007x over previous
"""TRN2 Bass kernel for nn_LogDomainResNet.

The reference network is a signed-log-domain encoding of a plain
real-domain tanh ResNet:

    v0      = sign_x * exp(log_abs_x)
    v_{i+1} = tanh(v_i @ W_i + b_i) + v_i        (7 inner layers)
    t       = v_7 @ W_final
    out     = stack([sign(t), log|t|])

All slog plumbing (per-row max, exp/log per layer) cancels exactly, so the
kernel computes in the real domain. Values stay bounded (|v| < 32), so fp32
range is never an issue.

Precision (1.5-pass scheme): each matmul runs as one fp16 pass plus one
fp8e4 DoubleRow correction accumulated into the same PSUM group.
  main:  vh @ (Wh * 2^12)            vh = fp16(v), Wh = fp16(W)
  corr:  (vl*2^8) @ (Wh8*2^4)  +  vh8 @ (Wl8*2^12)     [one DR instr/chunk]
All terms carry 2^12, so tanh reads PSUM with scale=2^-12. Effective
operand precision ~15-16 bits; the exact v is carried in an fp32 residual
tensor vf (updated in place), from which vh/vl are re-derived each layer.

Layout: activations live transposed ([feature -> partitions, batch -> free]);
the host precomputes v0's whole operand set (vh fp16 / vl bf16 / fp8 lane
pair, pre-transposed) and the bias in its SBUF layout, so the kernel has no
input stage at all — layer-0 weights and the v0 set stream in
chunk-interleaved and matmuls start immediately.  w8 lane 0 is derived
on-chip from the fp16 weights on the otherwise idle GpSimd engine.  The
final layer swaps operands (stationary = vh tile) to produce t in natural
[batch, feature] layout (sign/abs as DVE bitwise ops), so outputs DMA out
contiguously.

Sharding: data-parallel over the batch axis, 1024 rows per core x 8 cores.
"""

import numpy as np
import ml_dtypes

_B, _D, _NL = 8192, 1024, 8  # batch, width, layers (7 inner + final)
_NCORES = 8
_BP = _B // _NCORES          # batch rows per core
_P = 128
_KC = _D // _P               # contraction chunks per matmul
_BT = _BP // _P              # batch tiles (final stage)
_BCH = 512                   # PSUM free dim
_NBC = _BP // _BCH           # batch chunks per layer pass
_NT = _D // _P               # out-feature tiles per layer
_SC = 4096.0                 # 2^12 PSUM scale carried by both passes

_cached_nc = None
last_results = None  # BassKernelResults from the most recent run (for test.py)


def _build():
    import concourse.mybir as mybir
    from concourse import bacc
    from concourse.tile import TileContext

    f32, f16 = mybir.dt.float32, mybir.dt.float16
    bf16, f8 = mybir.dt.bfloat16, mybir.dt.float8e4
    i32 = mybir.dt.int32
    AF = mybir.ActivationFunctionType
    ALU = mybir.AluOpType
    DR = mybir.MatmulPerfMode.DoubleRow

    nc = bacc.Bacc("TRN2", target_bir_lowering=False, debug=False)
    # host-prepared v0 operand set, pre-transposed to [D, BP] / [D, 2, BP]
    d_vl = nc.dram_tensor("vl0", [_D, _BP], bf16, kind="ExternalInput")
    d_vh = nc.dram_tensor("vh0", [_D, _BP], f16, kind="ExternalInput")
    d_v8 = nc.dram_tensor("v80", [_D, 2, _BP], f8, kind="ExternalInput")
    d_wh = nc.dram_tensor("wh", [_NL, _D, _D], f16, kind="ExternalInput")
    d_w8 = nc.dram_tensor("w8l", [_NL, _D, _D], f8, kind="ExternalInput")
    d_bias = nc.dram_tensor("bias", [_P, (_NL - 1) * _NT], f32, kind="ExternalInput")
    d_out = nc.dram_tensor("out", [2, _BP, _D], f32, kind="ExternalOutput")

    with TileContext(nc) as tc:
        with (
            tc.tile_pool(name="const", bufs=1) as constp,
            tc.tile_pool(name="w", bufs=2) as wp,
            tc.tile_pool(name="w8", bufs=2) as w8p,
            tc.tile_pool(name="vh", bufs=2) as vhp,
            tc.tile_pool(name="v8", bufs=2) as v8p,
            tc.tile_pool(name="vf", bufs=1) as vfp,
            tc.tile_pool(name="inp", bufs=1) as inp,
            tc.tile_pool(name="tmp", bufs=4) as tmp,
            tc.tile_pool(name="ps", bufs=4, space="PSUM") as ps,
        ):
            bias_sb = constp.tile([_P, (_NL - 1) * _NT], f32)

            # ---- input: v0 operand set DMA'd straight in (host-prepared) ----
            vf = vfp.tile([_P, _KC, _BP], f32, tag="vf")
            vh = vhp.tile([_P, _KC, _BP], f16, tag="vh")
            v8 = v8p.tile([_P, _KC, 2, _BP], f8, tag="v8")
            # layer-0 weights interleaved with the v0 set, chunk by chunk, so
            # the first matmuls start as soon as chunk 0 lands.  w8 lane 0
            # (Wh8 = fp8(Wh * 2^4)) is derived on-chip from the fp16 weights;
            # only lane 1 (Wl8) ships from the host.
            ws0 = wp.tile([_P, _KC, _D], f16, tag="w", name="ws0")
            w8s0 = w8p.tile([_P, _KC, 2, _D], f8, tag="w8", name="w8s0")
            vl0 = inp.tile([_P, _KC, _BP], bf16, tag="vl0")
            nc.sync.dma_start(bias_sb[:], d_bias[:, :])
            for c in range(_KC):
                csl = slice(c * _P, (c + 1) * _P)
                nc.sync.dma_start(ws0[:, c, :], d_wh[0, csl, :])
                nc.sync.dma_start(vh[:, c, :], d_vh[csl, :])
                if c == 0:
                    # PE warm-up: the HAM clock gate only reaches full rate
                    # after ~3us of continuous busy; layer 0 is DMA-paced, so
                    # prime the ramp with throwaway matmuls on the first
                    # weight chunk (result never read)
                    warm = ps.tile([_P, _BCH], f32, tag="warm", name="warmps")
                    for wi in range(12):
                        nc.tensor.matmul(
                            warm[:], ws0[:, 0, :_P], ws0[:, 0, :_BCH],
                            start=True, stop=True,
                        )
                nc.sync.dma_start(v8[:, c, :, :], d_v8[csl, :, :])
                nc.sync.dma_start(w8s0[:, c, 1, :], d_w8[0, csl, :])
                if c >= _KC // 2:
                    # trailing half: also pull in vl0 for the early epilogues
                    # (they gate PSUM-bank recycling in layer 0)
                    vc = c - _KC // 2
                    vcsl = slice(vc * _P, (vc + 1) * _P)
                    nc.sync.dma_start(vl0[:, vc, :], d_vl[vcsl, :])
                nc.gpsimd.tensor_scalar_mul(
                    out=w8s0[:, c, 0, :], in0=ws0[:, c, :], scalar1=0.00390625
                )
            for c in range(_KC // 2, _KC):
                csl = slice(c * _P, (c + 1) * _P)
                nc.sync.dma_start(vl0[:, c, :], d_vl[csl, :])

            # ---- 7 inner layers: v = tanh(v @ W + b) + v ----
            for i in range(_NL - 1):
                if i == 0:
                    ws, w8s = ws0, w8s0
                else:
                    ws = wp.tile([_P, _KC, _D], f16, tag="w")
                    nc.sync.dma_start(
                        ws[:], d_wh[i].rearrange("(c p) n -> p c n", p=_P)
                    )
                    w8s = w8p.tile([_P, _KC, 2, _D], f8, tag="w8")
                    nc.sync.dma_start(
                        w8s[:, :, 1, :],
                        d_w8[i].rearrange("(c p) n -> p c n", p=_P),
                    )
                    for c in range(_KC):
                        nc.gpsimd.tensor_scalar_mul(
                            out=w8s[:, c, 0, :], in0=ws[:, c, :], scalar1=0.00390625
                        )
                vh_new = vhp.tile([_P, _KC, _BP], f16, tag="vh")
                v8_new = v8p.tile([_P, _KC, 2, _BP], f8, tag="v8")
                for n in range(_NT):
                    nsl = slice(n * _P, (n + 1) * _P)
                    pts = [
                        ps.tile([_P, _BCH], f32, tag="mm", name=f"pt{i}_{n}_{bc}")
                        for bc in range(_NBC)
                    ]
                    # last group: finish bc0 (matmuls + epilogue) before bc1's
                    # matmuls so the next layer's last-needed chunk is ready
                    # sooner; other groups interleave bc to share DR weights
                    bc_waves = (
                        [[bc] for bc in range(_NBC)]
                        if n == _NT - 1 else [list(range(_NBC))]
                    )
                    for wave in bc_waves:
                        for c in range(_KC):
                            for bc in wave:
                                bsl = slice(bc * _BCH, (bc + 1) * _BCH)
                                nc.tensor.matmul(
                                    pts[bc][:], ws[:, c, nsl], vh[:, c, bsl],
                                    start=(c == 0), stop=False,
                                )
                        for c in range(_KC):
                            for bc in wave:
                                bsl = slice(bc * _BCH, (bc + 1) * _BCH)
                                nc.tensor.matmul(
                                    pts[bc][:], w8s[:, c, :, nsl], v8[:, c, :, bsl],
                                    start=False, stop=(c == _KC - 1),
                                    perf_mode=DR,
                                )
                        for bc in wave:
                            bsl = slice(bc * _BCH, (bc + 1) * _BCH)
                            u = tmp.tile([_P, _BCH], f32, tag="u", bufs=3)
                            nc.scalar.activation(
                                u[:], pts[bc][:], AF.Tanh, scale=1.0 / _SC,
                                bias=bias_sb[:, i * _NT + n : i * _NT + n + 1],
                            )
                            if i == 0:
                                # lazy vf init: vf = (u + vh0) + vl0
                                nc.vector.tensor_add(
                                    out=u[:], in0=u[:], in1=vh[:, n, bsl]
                                )
                                nc.vector.tensor_add(
                                    out=vf[:, n, bsl], in0=u[:], in1=vl0[:, n, bsl]
                                )
                            else:
                                nc.vector.tensor_add(
                                    out=vf[:, n, bsl], in0=u[:], in1=vf[:, n, bsl]
                                )
                            nc.scalar.activation(
                                vh_new[:, n, bsl], vf[:, n, bsl], AF.Identity
                            )
                            vlt = tmp.tile([_P, _BCH], bf16, tag="vl", bufs=3)
                            nc.vector.tensor_sub(
                                out=vlt[:], in0=vf[:, n, bsl], in1=vh_new[:, n, bsl]
                            )
                            nc.gpsimd.tensor_scalar_mul(
                                out=v8_new[:, n, 0, bsl], in0=vlt[:], scalar1=256.0
                            )
                            nc.scalar.activation(
                                v8_new[:, n, 1, bsl], vf[:, n, bsl], AF.Identity
                            )
                vh, v8 = vh_new, v8_new

            # ---- final layer: t = v @ W_f, out = [sign(t), log|t|] ----
            wf = wp.tile([_P, _KC, _D], f16, tag="w")
            nc.sync.dma_start(
                wf[:], d_wh[_NL - 1].rearrange("(c p) n -> p c n", p=_P)
            )
            w8f = w8p.tile([_P, _KC, 2, _D], f8, tag="w8")
            nc.sync.dma_start(
                w8f[:, :, 1, :],
                d_w8[_NL - 1].rearrange("(c p) n -> p c n", p=_P),
            )
            for c in range(_KC):
                nc.gpsimd.tensor_scalar_mul(
                    out=w8f[:, c, 0, :], in0=wf[:, c, :], scalar1=0.00390625
                )
            for bt in range(_BT):
                bsl = slice(bt * _P, (bt + 1) * _P)
                ptf = [
                    ps.tile([_P, _BCH], f32, tag="mm", name=f"ptf{bt}_{j}")
                    for j in range(_NBC)
                ]
                # j-interleaved so each (stationary) v-side tile is loaded
                # into the PE once per chunk, serving both j halves; the last
                # batch tile de-interleaves so j0 drains while j1 computes
                j_waves = (
                    [[j] for j in range(_NBC)]
                    if bt == _BT - 1 else [list(range(_NBC))]
                )
                for wave in j_waves:
                    for c in range(_KC):
                        for j in wave:
                            nsl = slice(j * _BCH, (j + 1) * _BCH)
                            nc.tensor.matmul(
                                ptf[j][:], vh[:, c, bsl], wf[:, c, nsl],
                                start=(c == 0), stop=False,
                            )
                    for c in range(_KC):
                        for j in wave:
                            nsl = slice(j * _BCH, (j + 1) * _BCH)
                            nc.tensor.matmul(
                                ptf[j][:], v8[:, c, :, bsl], w8f[:, c, :, nsl],
                                start=False, stop=(c == _KC - 1),
                                perf_mode=DR,
                            )
                    for j in wave:
                        nsl = slice(j * _BCH, (j + 1) * _BCH)
                        # sign/abs as DVE bitwise ops (sign = msb | 1.0f, abs
                        # = clear msb) so only the Ln runs on ACT — the
                        # trailing per-tile epilogue chain halves
                        sg = tmp.tile([_P, _BCH], f32, tag="sg", bufs=2)
                        if bt == _BT - 1:
                            # tail tiles: sign on ACT so DVE only does abs and
                            # the Ln chain starts one DVE-op sooner
                            nc.scalar.activation(sg[:], ptf[j][:], AF.Sign)
                        else:
                            nc.vector.tensor_scalar(
                                out=sg[:].bitcast(i32), in0=ptf[j][:].bitcast(i32),
                                scalar1=-0x80000000, scalar2=0x3F800000,
                                op0=ALU.bitwise_and, op1=ALU.bitwise_or,
                            )
                        ab = tmp.tile([_P, _BCH], f32, tag="ab", bufs=2)
                        nc.vector.tensor_scalar(
                            out=ab[:].bitcast(i32), in0=ptf[j][:].bitcast(i32),
                            scalar1=0x7FFFFFFF, scalar2=None,
                            op0=ALU.bitwise_and,
                        )
                        lg = tmp.tile([_P, _BCH], f32, tag="lg", bufs=2)
                        nc.scalar.activation(lg[:], ab[:], AF.Ln, scale=1.0 / _SC)
                        nc.sync.dma_start(d_out[0, bsl, nsl], sg[:])
                        nc.sync.dma_start(d_out[1, bsl, nsl], lg[:])
    nc.compile()
    return nc


def kernel(sign_x, log_abs_x, inner_kernels, final_kernel):
    global _cached_nc, last_results
    from concourse.bass_utils import run_bass_kernel_spmd

    if _cached_nc is None:
        _cached_nc = _build()
    nc = _cached_nc

    sign_x = np.asarray(sign_x, dtype=np.float32)
    log_abs_x = np.asarray(log_abs_x, dtype=np.float32)
    ik = np.asarray(inner_kernels, dtype=np.float32)
    fk = np.asarray(final_kernel, dtype=np.float32)
    e4 = ml_dtypes.float8_e4m3

    # host-side v0 operand set, transposed to [D, B]
    v0 = (sign_x * np.exp(log_abs_x)).T.astype(np.float32)
    vh0 = v0.astype(np.float16)
    vl0f = v0 - vh0.astype(np.float32)
    vl0 = vl0f.astype(ml_dtypes.bfloat16)
    v80 = np.empty((_D, 2, _B), dtype=e4)
    v80[:, 0, :] = np.clip(vl0f * 256.0, -240, 240).astype(e4)
    v80[:, 1, :] = np.clip(v0, -240, 240).astype(e4)

    W = np.concatenate([ik[:, :_D, :], fk[None]], axis=0)  # [8, 1024, 1024]
    Wh = W.astype(np.float16)
    Wl = W - Wh.astype(np.float32)
    w8l = np.ascontiguousarray(np.clip(Wl * _SC, -240, 240).astype(e4))
    WhS = np.ascontiguousarray(Wh.astype(np.float32) * _SC).astype(np.float16)
    # bias pre-arranged to the SBUF layout [p, (l t)] = [128, 56]
    bias = np.ascontiguousarray(
        ik[:, _D, :].reshape(_NL - 1, _NT, _P).transpose(2, 0, 1).reshape(_P, -1)
    )

    in_maps = []
    for cid in range(_NCORES):
        sl = slice(cid * _BP, (cid + 1) * _BP)
        in_maps.append({
            "vl0": np.ascontiguousarray(vl0[:, sl]),
            "vh0": np.ascontiguousarray(vh0[:, sl]),
            "v80": np.ascontiguousarray(v80[:, :, sl]),
            "wh": WhS,
            "w8l": w8l,
            "bias": bias,
        })

    last_results = run_bass_kernel_spmd(nc, in_maps, core_ids=list(range(_NCORES)))
    return np.concatenate([r["out"] for r in last_results.results], axis=1)
